# revision 19
# baseline (speedup 1.0000x reference)
"""Batched int8-valued GEMM with dequant epilogue on 8 Trainium2 NeuronCores.

Problem: a[64,1024,128] i32 (vals 0..126), b[64,1024,128] i32 (vals 0..126),
alpha[1] f32.  out[bt,m,n] = fp16(alpha * sum_k a[bt,m,k]*b[bt,n,k]).

Sharding: pure batch-parallel — 8 batches per core, no communication.

Design (per core; HBM-traffic-minimal, trace-driven):
  - Host prep: int32 inputs are narrowed to int8 (values 0..126, exact) and
    relayed out p-major [p, ib, t, k] (row m = 8p+t), so input HBM traffic
    is 2.1 MB/core (vs 8.4 as int32) and the whole tensor loads as one
    contiguous 8 KiB/partition descriptor chain (per-batch int8 loads were
    descriptor-rate-limited at ~180 GB/s).
  - SWDGE cast-DMAs int8 HBM -> bf16 SBUF; all inputs SBUF-resident
    (32 KiB/partition).  b0 halves + a0 first (compute fill), then the
    gpsimd const setup, then one big load per tensor for batches 1-7.
  - PE transposes (x identity, 4 tiles -> one bf16 PSUM bank) put K on
    partitions.  alpha is folded into the b-transpose copies
    (tensor_scalar_mul, 2x_1P mode — same cost as the plain copy), so the
    epilogue is a pure f32->fp16 copy.
  - Matmul pairs [128k x 128m] . [128k x 512n] -> [128,1024] f32 PSUM
    (2 banks; 16-bit PSUM matmul dst is TRN3-only); K=128 -> acc <= 2.03e6
    < 2^24 exact in f32; alpha*b in bf16 adds ~2^-9 relative error, far
    inside the 2e-2 gate (measured rel err 3.4e-4).
  - Epilogue: 8 PSUM->SBUF copies per batch, 5 on ACT (1.2 GHz) / 3 on DVE
    (0.96 GHz), permuted read AP undoing the b row interleave; transpose
    copies ride DVE.  Cadence ~5.4 us/batch ~= the ~358 GB/s per-core HBM
    share (716 GB/s per NC pair), so production and store drain stay
    balanced.
  - Stores: 2 x 1 MiB HWDGE stores per batch (8 KiB contiguous per
    partition); first and last halves split into 512 KiB quarters to start
    the stream sooner / shorten the final drain.  outp ring bufs=6 covers
    store drain + ~2us HBM completion latency so the epilogue->psm->PE
    chain never backs up (a bufs=3 ring caused a 4.4us PE stall + HAM
    re-throttle).  Next batch's b transposes spliced between halves keep
    the PE dense (HAM stays at 2.4 GHz).
"""

import numpy as np

B, M, N, K = 64, 1024, 1024, 128
NCORES = 8
BPC = B // NCORES  # batches per core
TM = M // 128  # m tiles per batch (8)
TN = N // 128  # n tiles per batch (8)

_CACHE = {}


def _build_module():
    from contextlib import ExitStack

    import concourse.tile as tile
    from concourse import bacc, mybir
    from concourse.bass import ds, ts
    from concourse.masks import make_identity

    fp16 = mybir.dt.float16
    bf16 = mybir.dt.bfloat16
    f32 = mybir.dt.float32
    i8 = mybir.dt.int8

    nc = bacc.Bacc("TRN2", debug=False, enable_asserts=False)
    # inputs are int8-valued (0..126): upload as int8 (host-side exact
    # narrowing) so the HBM read side is 1 B/elem instead of 4 B/elem —
    # input HBM traffic drops 8.39 MB -> 2.10 MB per core.  Host also
    # pre-interleaves to p-major [p, ib, t, k] (row m = 8p+t) so each
    # partition's data for ALL batches is one contiguous DRAM run: the
    # whole tensor loads in one descriptor chain (8 KB/partition) instead
    # of 16 descriptor-rate-limited per-batch loads.
    a_d = nc.dram_tensor("a", [128, BPC * M * K // 128], i8, kind="ExternalInput")
    b_d = nc.dram_tensor("b", [128, BPC * N * K // 128], i8, kind="ExternalInput")
    al_d = nc.dram_tensor("alpha", [1], f32, kind="ExternalInput")
    o_d = nc.dram_tensor("out", [BPC, M, N], fp16, kind="ExternalOutput")

    with ExitStack() as ctx:
        tc = ctx.enter_context(tile.TileContext(nc))
        const = ctx.enter_context(tc.tile_pool(name="const", bufs=1))
        inp = ctx.enter_context(tc.tile_pool(name="inp", bufs=1))
        io = ctx.enter_context(tc.tile_pool(name="io", bufs=3))
        # 6 x 1 MiB staged output halves: the elastic buffer must cover the
        # ~3us store drain + ~2us HBM write-completion (sem) latency at the
        # ~2.9us/half production cadence, or the epilogue->psm->PE chain
        # backs up, PE idles >3.4us and HAM re-throttles (seen in the v3
        # trace as a 4.4us PE stall + cold-clock restart).
        outp = ctx.enter_context(tc.tile_pool(name="outp", bufs=6))
        pst = ctx.enter_context(tc.tile_pool(name="pst", bufs=2, space="PSUM"))
        psm = ctx.enter_context(tc.tile_pool(name="psm", bufs=3, space="PSUM"))

        # Both inputs live SBUF-resident as one [128, 8*1024] bf16 tile each;
        # batch ib is the [:, ib*1024:(ib+1)*1024] slice (columns t*128+k).
        # Loads: batch-0 pieces first (split b0 so the first transpose group
        # starts sooner), then the gpsimd const setup (identity must not sit
        # behind the bulk emission), then one big load each for batches 1-7.
        a_all = inp.tile([128, BPC * M * K // 128], bf16, tag="a_all")
        b_all = inp.tile([128, BPC * N * K // 128], bf16, tag="b_all")

        nc.gpsimd.dma_start(b_all[:, ds(0, 512)], b_d.ap()[:, ds(0, 512)])
        nc.gpsimd.dma_start(b_all[:, ds(512, 512)], b_d.ap()[:, ds(512, 512)])
        nc.gpsimd.dma_start(a_all[:, ds(0, 1024)], a_d.ap()[:, ds(0, 1024)])

        ident = const.tile([128, 128], bf16)
        make_identity(nc, ident)
        alpha_1 = const.tile([1, 1], f32)
        nc.sync.dma_start(alpha_1[:], al_d.ap().rearrange("(a x) -> a x", a=1))
        ones_row = const.tile([1, 128], f32)
        nc.gpsimd.memset(ones_row[:], 1.0)
        # alpha broadcast to [128,1] via PE: ones_row.T @ alpha (contraction=1)
        alpha_ps = pst.tile([128, 1], f32, tag="ps")
        nc.tensor.matmul(alpha_ps[:], ones_row[:], alpha_1[:], start=True, stop=True)
        alpha_bc = const.tile([128, 1], f32)
        nc.vector.tensor_copy(alpha_bc[:], alpha_ps[:])

        # per-batch loads: the cast-DMA is bf16-write-side limited either way,
        # but per-batch completion sems let batch k's transposes start as soon
        # as its own 256 KiB lands (a single big load's sem fires only at the
        # very end — measured +4us)
        for ib in range(1, BPC):
            nc.gpsimd.dma_start(
                b_all[:, ds(ib * 1024, 1024)], b_d.ap()[:, ds(ib * 1024, 1024)]
            )
            nc.gpsimd.dma_start(
                a_all[:, ds(ib * 1024, 1024)], a_d.ap()[:, ds(ib * 1024, 1024)]
            )

        in_tiles = {
            ib: (a_all[:, ds(ib * 1024, 1024)], b_all[:, ds(ib * 1024, 1024)])
            for ib in range(BPC)
        }

        def transpose_group(src_bf, dst_T, g, scale=False):
            """PE-transpose 4 [128,128] tiles of src into one bf16 psum bank,
            then one DVE copy into dst_T[:, g*512:(g+1)*512].  scale=True
            multiplies by alpha on the way out (b side)."""
            ps = pst.tile([128, 512], bf16, tag="ps")
            for q in range(4):
                t = 4 * g + q
                nc.tensor.transpose(
                    ps[:, ts(q, 128)], src_bf[:, ts(t, 128)], ident[:]
                )
            dst = dst_T[:, ds(g * 512, 512)]
            if scale:
                nc.vector.tensor_scalar_mul(dst, ps[:], alpha_bc[:])
            else:
                nc.vector.tensor_copy(dst, ps[:])

        # bT is produced one batch ahead so the b transposes interleave with
        # the previous batch's matmul stream (PE stays dense -> HAM warm).
        bT_next = io.tile([128, N], bf16, tag="bT")
        transpose_group(in_tiles[0][1], bT_next, 0, scale=True)
        transpose_group(in_tiles[0][1], bT_next, 1, scale=True)

        for ib in range(BPC):
            a_bf, _ = in_tiles[ib]  # b transposed one iteration ahead

            # aT/bT: [k, j] with j = t*128 + p  <->  row index 8p + t
            aT = io.tile([128, M], bf16, tag="aT")
            bT = bT_next
            if ib + 1 < BPC:
                bT_next = io.tile([128, N], bf16, tag="bT")

            for half in range(2):
                transpose_group(a_bf, aT, half)
                out_sb = outp.tile([128, M * N // 128 // 2], fp16, tag="out_sb")
                for u in range(2):
                    for tt in range(2):
                        t = 4 * half + 2 * u + tt
                        e = 4 * half + 2 * u + tt  # epilogue index 0..7
                        ps = psm.tile([128, 1024], f32)
                        for nh in range(2):
                            nc.tensor.matmul(
                                ps[:, ds(nh * 512, 512)],
                                aT[:, ts(t, 128)],
                                bT[:, ds(nh * 512, 512)],
                                start=True,
                                stop=True,
                            )
                        # psum free j = t'*128 + p <-> n = 8p + t'; read in
                        # n order: outer p (stride 1, x128), inner t'
                        # (stride 128, x8)
                        ps_n_order = ps[:].rearrange("p (t q) -> p q t", t=8)
                        o_slice = out_sb[
                            :, ds((2 * u + tt) * N, N)
                        ].rearrange("p (q t) -> p q t", t=8)
                        # 5 ACT / 3 DVE per batch: ACT (1.2 GHz) is faster
                        # per op, DVE also carries the 4 transpose copies.
                        if e in (1, 3, 5):
                            nc.vector.tensor_copy(o_slice, ps_n_order)
                        else:
                            nc.scalar.copy(o_slice, ps_n_order)

                    # first half: quarter stores (512 KiB) so the HBM store
                    # stream starts ~2us sooner; last half: quarter stores so
                    # the final drain+completion tail is short
                    if (ib, half) in ((0, 0), (BPC - 1, 1)):
                        nc.sync.dma_start(
                            o_d.ap()[ib].rearrange("(p t) n -> p t n", t=TM)[
                                :, 4 * half + 2 * u : 4 * half + 2 * u + 2, :
                            ],
                            out_sb[:].rearrange("p (t n) -> p t n", n=N)[
                                :, 2 * u : 2 * u + 2, :
                            ],
                        )

                    # next batch's b transposes, spliced between mm quarters
                    if half == 1 and ib + 1 < BPC:
                        transpose_group(
                            in_tiles[ib + 1][1], bT_next, u, scale=True
                        )

                # rows 8p + t, t in [4*half, 4*half+4): 8 KiB contiguous per
                # partition, 1 MiB per store
                if (ib, half) not in ((0, 0), (BPC - 1, 1)):
                    nc.sync.dma_start(
                        o_d.ap()[ib].rearrange("(p t) n -> p t n", t=TM)[
                            :, 4 * half : 4 * half + 4, :
                        ],
                        out_sb[:].rearrange("p (t n) -> p t n", n=N),
                    )

    nc.compile()
    return nc


def _get_module():
    if "nc" not in _CACHE:
        _CACHE["nc"] = _build_module()
    return _CACHE["nc"]


def run(a, b, alpha, trace=False, **kw):
    from concourse.bass_utils import run_bass_kernel_spmd

    nc = _get_module()

    def prep(x):
        # values are 0..126: int8 narrowing is exact.  Relayout to p-major
        # [p, ib, t, k] (row m = 8p+t) so each partition's span for all 8
        # batches is one contiguous DRAM run per core.
        x = np.ascontiguousarray(x).astype(np.int8)
        x = x.reshape(NCORES, BPC, 128, TM, K)  # [core, ib, p, t, k]
        x = x.transpose(0, 2, 1, 3, 4)  # [core, p, ib, t, k]
        return np.ascontiguousarray(x.reshape(NCORES, 128, BPC * TM * K))

    a = prep(a)
    b = prep(b)
    alpha = np.ascontiguousarray(alpha, dtype=np.float32)
    in_maps = [
        {"a": a[i], "b": b[i], "alpha": alpha} for i in range(NCORES)
    ]
    res = run_bass_kernel_spmd(
        nc, in_maps, core_ids=list(range(NCORES)), trace=trace, **kw
    )
    out = np.concatenate([r["out"] for r in res.results], axis=0)
    return out, res


def kernel(a, b, alpha):
    out, _ = run(a, b, alpha, trace=False)
    return out


# revision 23
# speedup vs baseline: 1.0543x; 1.0543x over previous
"""Batched int8-valued GEMM with dequant epilogue on 8 Trainium2 NeuronCores.

Problem: a[64,1024,128] i32 (vals 0..126), b[64,1024,128] i32 (vals 0..126),
alpha[1] f32.  out[bt,m,n] = fp16(alpha * sum_k a[bt,m,k]*b[bt,n,k]).

Sharding: pure batch-parallel — 8 batches per core, no communication.

Design (per core; HBM-traffic-minimal, trace-driven):
  - Host prep: int32 inputs are narrowed to int8 (values 0..126, exact) and
    relayed out p-major [p, ib, t, k] (row m = 8p+t), so input HBM traffic
    is 2.1 MB/core (vs 8.4 as int32) and the whole tensor loads as one
    contiguous 8 KiB/partition descriptor chain (per-batch int8 loads were
    descriptor-rate-limited at ~180 GB/s).
  - SWDGE cast-DMAs int8 HBM -> bf16 SBUF; all inputs SBUF-resident
    (32 KiB/partition).  b0 halves + a0 first (compute fill), then the
    gpsimd const setup, then one big load per tensor for batches 1-7.
  - PE transposes (x identity, 4 tiles -> one bf16 PSUM bank) put K on
    partitions.  alpha is folded into the b-transpose copies
    (tensor_scalar_mul, 2x_1P mode — same cost as the plain copy), so the
    epilogue is a pure f32->fp16 copy.
  - Matmul pairs [128k x 128m] . [128k x 512n] -> [128,1024] f32 PSUM
    (2 banks; 16-bit PSUM matmul dst is TRN3-only); K=128 -> acc <= 2.03e6
    < 2^24 exact in f32; alpha*b in bf16 adds ~2^-9 relative error, far
    inside the 2e-2 gate (measured rel err 3.4e-4).
  - Epilogue: 8 PSUM->SBUF copies per batch, 5 on ACT (1.2 GHz) / 3 on DVE
    (0.96 GHz), permuted read AP undoing the b row interleave; transpose
    copies ride DVE.  Cadence ~5.4 us/batch ~= the ~358 GB/s per-core HBM
    share (716 GB/s per NC pair), so production and store drain stay
    balanced.
  - Stores: 2 x 1 MiB HWDGE stores per batch (8 KiB contiguous per
    partition); first and last halves split into 512 KiB quarters to start
    the stream sooner / shorten the final drain.  outp ring bufs=6 covers
    store drain + ~2us HBM completion latency so the epilogue->psm->PE
    chain never backs up (a bufs=3 ring caused a 4.4us PE stall + HAM
    re-throttle).  Next batch's b transposes spliced between halves keep
    the PE dense (HAM stays at 2.4 GHz).
"""

import numpy as np

B, M, N, K = 64, 1024, 1024, 128
NCORES = 8
BPC = B // NCORES  # batches per core
TM = M // 128  # m tiles per batch (8)
TN = N // 128  # n tiles per batch (8)

_CACHE = {}


def _build_module():
    from contextlib import ExitStack

    import concourse.tile as tile
    from concourse import bacc, mybir
    from concourse.bass import ds, ts
    from concourse.masks import make_identity

    fp16 = mybir.dt.float16
    bf16 = mybir.dt.bfloat16
    f32 = mybir.dt.float32
    i8 = mybir.dt.int8

    nc = bacc.Bacc("TRN2", debug=False, enable_asserts=False)
    # inputs are int8-valued (0..126): upload as int8 (host-side exact
    # narrowing) so the HBM read side is 1 B/elem instead of 4 B/elem —
    # input HBM traffic drops 8.39 MB -> 2.10 MB per core.  Host also
    # pre-interleaves to p-major [p, ib, t, k] (row m = 8p+t) so each
    # partition's data for ALL batches is one contiguous DRAM run: the
    # whole tensor loads in one descriptor chain (8 KB/partition) instead
    # of 16 descriptor-rate-limited per-batch loads.
    a_d = nc.dram_tensor("a", [128, BPC * M * K // 128], i8, kind="ExternalInput")
    b_d = nc.dram_tensor("b", [128, BPC * N * K // 128], i8, kind="ExternalInput")
    al_d = nc.dram_tensor("alpha", [1], f32, kind="ExternalInput")
    o_d = nc.dram_tensor("out", [BPC, M, N], fp16, kind="ExternalOutput")

    with ExitStack() as ctx:
        tc = ctx.enter_context(tile.TileContext(nc))
        const = ctx.enter_context(tc.tile_pool(name="const", bufs=1))
        inp = ctx.enter_context(tc.tile_pool(name="inp", bufs=1))
        io = ctx.enter_context(tc.tile_pool(name="io", bufs=3))
        # 8 x 1 MiB staged output halves: the elastic buffer must cover the
        # ~3us store drain + ~2us HBM write-completion (sem) latency at the
        # ~2.9us/half production cadence, or the epilogue->psm->PE chain
        # backs up, PE idles >3.4us and HAM re-throttles (seen in the v3
        # trace as a 4.4us PE stall + cold-clock restart).
        outp = ctx.enter_context(tc.tile_pool(name="outp", bufs=8))
        pst = ctx.enter_context(tc.tile_pool(name="pst", bufs=2, space="PSUM"))
        psm = ctx.enter_context(tc.tile_pool(name="psm", bufs=3, space="PSUM"))

        # Both inputs live SBUF-resident as one [128, 8*1024] bf16 tile each;
        # batch ib is the [:, ib*1024:(ib+1)*1024] slice (columns t*128+k).
        # Loads: batch-0 pieces first (split b0 so the first transpose group
        # starts sooner), then the gpsimd const setup (identity must not sit
        # behind the bulk emission), then one big load each for batches 1-7.
        a_all = inp.tile([128, BPC * M * K // 128], bf16, tag="a_all")
        b_all = inp.tile([128, BPC * N * K // 128], bf16, tag="b_all")

        nc.gpsimd.dma_start(b_all[:, ds(0, 512)], b_d.ap()[:, ds(0, 512)])
        nc.gpsimd.dma_start(b_all[:, ds(512, 512)], b_d.ap()[:, ds(512, 512)])
        nc.gpsimd.dma_start(a_all[:, ds(0, 1024)], a_d.ap()[:, ds(0, 1024)])

        ident = const.tile([128, 128], bf16)
        make_identity(nc, ident)

        # ~10 throwaway identity transposes keep the PE busy from ~9us so the
        # HAM clock gate opens (needs ~3.4us sustained activity) before the
        # first real matmuls — otherwise batch 0 runs at 1.2 GHz (634ns/MM).
        warm = pst.tile([128, 128], bf16, tag="ps")
        for _ in range(10):
            nc.tensor.transpose(warm[:], ident[:], ident[:])

        alpha_1 = const.tile([1, 1], f32)
        nc.sync.dma_start(alpha_1[:], al_d.ap().rearrange("(a x) -> a x", a=1))
        ones_row = const.tile([1, 128], f32)
        nc.gpsimd.memset(ones_row[:], 1.0)
        # alpha broadcast to [128,1] via PE: ones_row.T @ alpha (contraction=1)
        alpha_ps = pst.tile([128, 1], f32, tag="ps")
        nc.tensor.matmul(alpha_ps[:], ones_row[:], alpha_1[:], start=True, stop=True)
        alpha_bc = const.tile([128, 1], f32)
        nc.vector.tensor_copy(alpha_bc[:], alpha_ps[:])

        # per-batch loads: the cast-DMA is bf16-write-side limited either way,
        # but per-batch completion sems let batch k's transposes start as soon
        # as its own 256 KiB lands (a single big load's sem fires only at the
        # very end — measured +4us)
        for ib in range(1, BPC):
            nc.gpsimd.dma_start(
                b_all[:, ds(ib * 1024, 1024)], b_d.ap()[:, ds(ib * 1024, 1024)]
            )
            nc.gpsimd.dma_start(
                a_all[:, ds(ib * 1024, 1024)], a_d.ap()[:, ds(ib * 1024, 1024)]
            )

        in_tiles = {
            ib: (a_all[:, ds(ib * 1024, 1024)], b_all[:, ds(ib * 1024, 1024)])
            for ib in range(BPC)
        }

        def transpose_group(src_bf, dst_T, g, scale=False):
            """PE-transpose 4 [128,128] tiles of src into one bf16 psum bank,
            then one DVE copy into dst_T[:, g*512:(g+1)*512].  scale=True
            multiplies by alpha on the way out (b side)."""
            ps = pst.tile([128, 512], bf16, tag="ps")
            for q in range(4):
                t = 4 * g + q
                nc.tensor.transpose(
                    ps[:, ts(q, 128)], src_bf[:, ts(t, 128)], ident[:]
                )
            dst = dst_T[:, ds(g * 512, 512)]
            if scale:
                nc.vector.tensor_scalar_mul(dst, ps[:], alpha_bc[:])
            else:
                nc.vector.tensor_copy(dst, ps[:])

        # bT is produced one batch ahead so the b transposes interleave with
        # the previous batch's matmul stream (PE stays dense -> HAM warm).
        bT_next = io.tile([128, N], bf16, tag="bT")
        transpose_group(in_tiles[0][1], bT_next, 0, scale=True)
        transpose_group(in_tiles[0][1], bT_next, 1, scale=True)

        for ib in range(BPC):
            a_bf, _ = in_tiles[ib]  # b transposed one iteration ahead

            # aT/bT: [k, j] with j = t*128 + p  <->  row index 8p + t
            aT = io.tile([128, M], bf16, tag="aT")
            bT = bT_next
            if ib + 1 < BPC:
                bT_next = io.tile([128, N], bf16, tag="bT")

            for half in range(2):
                transpose_group(a_bf, aT, half)
                out_sb = outp.tile([128, M * N // 128 // 2], fp16, tag="out_sb")
                for u in range(2):
                    for tt in range(2):
                        t = 4 * half + 2 * u + tt
                        e = 4 * half + 2 * u + tt  # epilogue index 0..7
                        ps = psm.tile([128, 1024], f32)
                        for nh in range(2):
                            nc.tensor.matmul(
                                ps[:, ds(nh * 512, 512)],
                                aT[:, ts(t, 128)],
                                bT[:, ds(nh * 512, 512)],
                                start=True,
                                stop=True,
                            )
                        # psum free j = t'*128 + p <-> n = 8p + t'; read in
                        # n order: outer p (stride 1, x128), inner t'
                        # (stride 128, x8)
                        ps_n_order = ps[:].rearrange("p (t q) -> p q t", t=8)
                        o_slice = out_sb[
                            :, ds((2 * u + tt) * N, N)
                        ].rearrange("p (q t) -> p q t", t=8)
                        # 5 ACT / 3 DVE per batch: ACT (1.2 GHz) is faster
                        # per op, DVE also carries the 4 transpose copies.
                        if e in (1, 3, 5):
                            nc.vector.tensor_copy(o_slice, ps_n_order)
                        else:
                            nc.scalar.copy(o_slice, ps_n_order)

                    # first half: quarter stores (512 KiB) so the HBM store
                    # stream starts ~2us sooner; last half: quarter stores so
                    # the final drain+completion tail is short
                    if (ib, half) in ((0, 0), (BPC - 1, 1)):
                        nc.sync.dma_start(
                            o_d.ap()[ib].rearrange("(p t) n -> p t n", t=TM)[
                                :, 4 * half + 2 * u : 4 * half + 2 * u + 2, :
                            ],
                            out_sb[:].rearrange("p (t n) -> p t n", n=N)[
                                :, 2 * u : 2 * u + 2, :
                            ],
                        )

                    # next batch's b transposes, spliced between mm quarters
                    if half == 1 and ib + 1 < BPC:
                        transpose_group(
                            in_tiles[ib + 1][1], bT_next, u, scale=True
                        )

                # rows 8p + t, t in [4*half, 4*half+4): 8 KiB contiguous per
                # partition, 1 MiB per store.  Once the SWDGE load queue has
                # drained (~batch 3), alternate stores onto it as a second
                # DMA queue so one queue's completion bubbles are covered by
                # the other.
                if (ib, half) not in ((0, 0), (BPC - 1, 1)):
                    eng = nc.gpsimd if (ib >= 3 and half == 1) else nc.sync
                    eng.dma_start(
                        o_d.ap()[ib].rearrange("(p t) n -> p t n", t=TM)[
                            :, 4 * half : 4 * half + 4, :
                        ],
                        out_sb[:].rearrange("p (t n) -> p t n", n=N),
                    )

    nc.compile()
    return nc


def _get_module():
    if "nc" not in _CACHE:
        _CACHE["nc"] = _build_module()
    return _CACHE["nc"]


def run(a, b, alpha, trace=False, **kw):
    from concourse.bass_utils import run_bass_kernel_spmd

    nc = _get_module()

    def prep(x):
        # values are 0..126: int8 narrowing is exact.  Relayout to p-major
        # [p, ib, t, k] (row m = 8p+t) so each partition's span for all 8
        # batches is one contiguous DRAM run per core.
        x = np.ascontiguousarray(x).astype(np.int8)
        x = x.reshape(NCORES, BPC, 128, TM, K)  # [core, ib, p, t, k]
        x = x.transpose(0, 2, 1, 3, 4)  # [core, p, ib, t, k]
        return np.ascontiguousarray(x.reshape(NCORES, 128, BPC * TM * K))

    a = prep(a)
    b = prep(b)
    alpha = np.ascontiguousarray(alpha, dtype=np.float32)
    in_maps = [
        {"a": a[i], "b": b[i], "alpha": alpha} for i in range(NCORES)
    ]
    res = run_bass_kernel_spmd(
        nc, in_maps, core_ids=list(range(NCORES)), trace=trace, **kw
    )
    out = np.concatenate([r["out"] for r in res.results], axis=0)
    return out, res


def kernel(a, b, alpha):
    out, _ = run(a, b, alpha, trace=False)
    return out
